# revision 37
# baseline (speedup 1.0000x reference)
"""Trainium2 Bass kernel for nn_LQE (topk_masking).

out = scores + MLP(topk_softmax_stats(pred_corners))

Math notes:
- top_k(softmax(x)) == softmax values of top_k(x) (softmax is monotone), and
  exp(x) is safe unnormalized here (|x| <~ 6 for randn inputs), so
  p_k = exp(x)_k / sum(exp(x)).  DVE `max` returns top-8 sorted descending in
  one instruction -> top-4 directly.
- The mean-of-top4 stat feature folds into W1 on the host:
  W1eff[c*4+k] = W1[c*5+k] + 0.25*W1[c*5+4]; b2 folds into scores.

Engine budget per 1024-row supertile (steady state ~3.2us):
- DVE: the 32 max8 instructions (~3.0us, the bottleneck) + reciprocal of the
  denominators + the top4 * (1/s) scale.
- Pool (GPSIMD): the denominator pairwise add-tree (TensorTensor add is the
  only legal elementwise reduce on Pool) + 3 groups of the final
  scores+quality broadcast add.
- ACT: exp / relu / PSUM->SBUF copies.
- PE: stat transposes, both MLP layers, and scores+quality for 5 groups via
  PSUM accumulation (identity matmul + one-hot broadcast matmul).
All device I/O is fp16 to halve DMA_ENGINES occupancy; the fp16 rounding is
~5e-4 relative, far inside the 2e-2 gate.

The 39 supertiles are emitted as a 17-stage software pipeline with exactly
one cross-engine hop per stage boundary: at steady state every instruction's
producers completed >=1 tick earlier, so the in-order sequencers never
head-block (DVE streams max8 back-to-back).  A 64-row tail tile handles
40000 = 39*1024 + 64 exactly, its stages interleaved into the main ticks.

Sharding: pure data-parallel over B*L = 320000 rows, 40000 rows/core on 8
cores (row = 8*p + g within a supertile).
"""

import numpy as np

K = 4
C = 4
NB = 33
HID = 64
G = 8  # row-groups per partition per supertile
ROWS_PER_TILE = 128 * G  # 1024
N_CORES = 8
ROWS_PER_CORE = 40000
N_FULL = 39  # 39 * 1024 = 39936
TAIL = 64  # + 64 = 40000

_CACHE = {}


def _build(rows):
    import concourse.bacc as bacc
    import concourse.mybir as mybir
    from concourse.tile import TileContext

    f32 = mybir.dt.float32
    f16 = mybir.dt.float16
    assert rows == N_FULL * ROWS_PER_TILE + TAIL

    nc = bacc.Bacc("TRN2")
    pred = nc.dram_tensor("pred", [rows, C * NB], f16, kind="ExternalInput")
    scores = nc.dram_tensor("scores", [rows, 80], f16, kind="ExternalInput")
    # w1pair[:, pair*128 + col]: col<64 -> hidden col of even group of pair,
    # col>=64 -> hidden of odd group; rows j in [0,64): j//16 = subgroup,
    # j%16 = stat feature (c*4+k).  Zero rows select the right subgroup.
    w1pair = nc.dram_tensor("w1pair", [64, 256], f16, kind="ExternalInput")
    # w2q[hh, blk*8+j] = W2[hh%64] iff j == (blk//2)*4 + (blk%2)*2 + hh//64:
    # one-hot columns so 4 accumulating matmuls produce qcol[p, g] from the
    # 2-group-stacked hT.  Column 0 of block 0 (rows 0..63) is plain W2,
    # reused by the tail tile.
    w2q = nc.dram_tensor("w2q", [128, 32], f16, kind="ExternalInput")
    b1r = nc.dram_tensor("b1r", [128, 1], f32, kind="ExternalInput")
    ident = nc.dram_tensor("ident", [128, 128], f16, kind="ExternalInput")
    # mask8[g, j] = (j // 80 == g), for the one-hot broadcast matmul.
    mask8 = nc.dram_tensor("mask8", [8, 640], f16, kind="ExternalInput")
    out = nc.dram_tensor("out", [rows, 80], f16, kind="ExternalOutput")

    NG = G * C  # 32 softmax groups per partition
    PEG = 5  # groups 0..4 take the PE out-add path; 5..7 go to Pool

    with TileContext(nc) as tc:
        with (
            tc.tile_pool(name="singles", bufs=1) as singles,
            tc.tile_pool(name="pin", bufs=6) as pin,
            tc.tile_pool(name="sin", bufs=19) as sin,
            tc.tile_pool(name="epool", bufs=6) as epool,
            tc.tile_pool(name="tpool", bufs=6) as tpool,
            tc.tile_pool(name="small", bufs=5) as small,
            tc.tile_pool(name="statp", bufs=4) as statp,
            tc.tile_pool(name="stp", bufs=4) as stp,
            tc.tile_pool(name="hpool", bufs=4) as hpool,
            tc.tile_pool(name="qpool", bufs=6) as qpool,
            tc.tile_pool(name="opool", bufs=4) as opool,
            tc.tile_pool(name="ps_t", bufs=2, space="PSUM") as ps_t,
            tc.tile_pool(name="ps_h", bufs=2, space="PSUM") as ps_h,
            tc.tile_pool(name="ps_q", bufs=1, space="PSUM") as ps_q,
            tc.tile_pool(name="ps_qt", bufs=1, space="PSUM") as ps_qt,
            tc.tile_pool(name="ps_o", bufs=2, space="PSUM") as ps_o,
        ):
            w1_sb = singles.tile([64, 256], f16)
            nc.sync.dma_start(out=w1_sb, in_=w1pair[:, :])
            w2_sb = singles.tile([128, 32], f16)
            nc.sync.dma_start(out=w2_sb, in_=w2q[:, :])
            b1_sb = singles.tile([128, 1], f32)
            nc.sync.dma_start(out=b1_sb, in_=b1r[:, :])
            ident_sb = singles.tile([128, 128], f16)
            nc.sync.dma_start(out=ident_sb, in_=ident[:, :])
            mask_sb = singles.tile([8, 640], f16)
            nc.sync.dma_start(out=mask_sb, in_=mask8[:, :])

            tctx = {}

            def st0(t):  # loads (row = G*p + g) on SP
                r0 = t * ROWS_PER_TILE
                d = {}
                hp = tc.high_priority()
                hp.__enter__()
                d["pred_t"] = pin.tile([128, G, C * NB], f16, name="pred_t")
                nc.sync.dma_start(
                    out=d["pred_t"],
                    in_=pred[r0 : r0 + ROWS_PER_TILE, :].rearrange(
                        "(p g) d -> p g d", p=128
                    ),
                )
                d["scores_t"] = sin.tile([128, G, 80], f16, name="scores_t")
                nc.sync.dma_start(
                    out=d["scores_t"],
                    in_=scores[r0 : r0 + ROWS_PER_TILE, :].rearrange(
                        "(p g) d -> p g d", p=128
                    ),
                )
                hp.__exit__(None, None, None)
                tctx[t] = d

            def st1(t):  # e = exp(pred) on ACT (fp16 out)
                d = tctx[t]
                d["e"] = epool.tile([128, G, C * NB], f16, name="e")
                with tc.high_priority():
                    nc.scalar.activation(
                        out=d["e"],
                        in_=d["pred_t"],
                        func=mybir.ActivationFunctionType.Exp,
                    )

            def st2(t):  # top-8 per group on DVE; first denominator add
                # level (c16 = e[0:16] + e[16:32]) on Pool
                d = tctx[t]
                e = d["e"]
                ef = e.rearrange("p g d -> p (g d)")
                t8 = tpool.tile([128, NG, 8], f16)
                t8f = t8.rearrange("p i k -> p (i k)")
                for i in range(NG):
                    nc.vector.max(
                        out=t8f[:, i * 8 : i * 8 + 8],
                        in_=ef[:, i * NB : (i + 1) * NB],
                    )
                e3 = e.rearrange("p g (c b) -> p (g c) b", b=NB)
                c16 = small.tile([128, NG, 16], f16, name="c16")
                nc.gpsimd.tensor_tensor(
                    out=c16,
                    in0=e3[:, :, 0:16],
                    in1=e3[:, :, 16:32],
                    op=mybir.AluOpType.add,
                )
                d["t8"] = t8
                d["c16"] = c16
                d["e3"] = e3

            def st3(t):  # rest of the denominator add-tree on Pool
                d = tctx[t]
                c16 = d["c16"]
                c8 = small.tile([128, NG, 8], f16, name="c8")
                nc.gpsimd.tensor_tensor(
                    out=c8,
                    in0=c16[:, :, 0:8],
                    in1=c16[:, :, 8:16],
                    op=mybir.AluOpType.add,
                )
                c4 = small.tile([128, NG, 4], f16, name="c4")
                nc.gpsimd.tensor_tensor(
                    out=c4,
                    in0=c8[:, :, 0:4],
                    in1=c8[:, :, 4:8],
                    op=mybir.AluOpType.add,
                )
                c2 = small.tile([128, NG, 2], f16, name="c2")
                nc.gpsimd.tensor_tensor(
                    out=c2,
                    in0=c4[:, :, 0:2],
                    in1=c4[:, :, 2:4],
                    op=mybir.AluOpType.add,
                )
                s1 = small.tile([128, NG], f16, name="s1")
                nc.gpsimd.tensor_tensor(
                    out=s1,
                    in0=c2[:, :, 0],
                    in1=c2[:, :, 1],
                    op=mybir.AluOpType.add,
                )
                s32 = small.tile([128, NG], f16, name="s32")
                nc.gpsimd.tensor_tensor(
                    out=s32,
                    in0=s1,
                    in1=d["e3"][:, :, 32],
                    op=mybir.AluOpType.add,
                )
                d["s32"] = s32

            def st4(t):  # r32 = 1/s on DVE
                d = tctx[t]
                d["r32"] = small.tile([128, NG], f16, name="r32")
                with nc.allow_low_precision(reason="fp16 stat tolerates 5e-4"):
                    nc.vector.reciprocal(out=d["r32"], in_=d["s32"])

            def st5(t):  # stat = top4 * (1/s) on DVE (fp16 2x)
                d = tctx[t]
                stat = statp.tile([128, NG, K], f16)
                nc.vector.tensor_tensor(
                    out=stat,
                    in0=d["t8"][:, :, 0:K],
                    in1=d["r32"][:].unsqueeze(2).broadcast_to([128, NG, K]),
                    op=mybir.AluOpType.mult,
                )
                d["stat"] = stat

            def st6(t):  # statT transposes on PE (two 64-col halves)
                d = tctx[t]
                statf = d["stat"].rearrange("p i k -> p (i k)")
                statT = ps_t.tile([64, 256], f16)
                nc.tensor.transpose(
                    out=statT[:, 0:128], in_=statf[:, 0:64], identity=ident_sb
                )
                nc.tensor.transpose(
                    out=statT[:, 128:256], in_=statf[:, 64:128], identity=ident_sb
                )
                d["statT"] = statT

            def st7(t):  # statT PSUM -> SBUF on ACT
                d = tctx[t]
                d["statT_sb"] = stp.tile([64, 256], f16, name="statT_sb")
                nc.scalar.copy(out=d["statT_sb"], in_=d["statT"])

            def st8(t):  # MLP layer 1 on PE, two groups stacked per matmul
                # hT[hh, blk*128+p]: group = (blk//2)*4 + (blk%2)*2 + hh//64.
                d = tctx[t]
                hT = ps_h.tile([128, 4 * 128], f32)
                for half in range(2):
                    for pairidx in range(2):
                        blk = half * 2 + pairidx
                        nc.tensor.matmul(
                            out=hT[:, blk * 128 : (blk + 1) * 128],
                            lhsT=w1_sb[:, pairidx * 128 : (pairidx + 1) * 128],
                            rhs=d["statT_sb"][:, half * 128 : (half + 1) * 128],
                            start=True,
                            stop=True,
                        )
                d["hT"] = hT

            def st9(t):  # relu + b1 on ACT
                d = tctx[t]
                d["hT_sb"] = hpool.tile([128, 4 * 128], f16, name="hT_sb")
                nc.scalar.activation(
                    out=d["hT_sb"],
                    in_=d["hT"],
                    func=mybir.ActivationFunctionType.Relu,
                    bias=b1_sb,
                    scale=1.0,
                )

            def st10(t):  # MLP layer 2 on PE: qcol[p, g] via one-hot accum
                d = tctx[t]
                qcol = ps_q.tile([128, G], f32)
                for blk in range(4):
                    nc.tensor.matmul(
                        out=qcol,
                        lhsT=d["hT_sb"][:, blk * 128 : (blk + 1) * 128],
                        rhs=w2_sb[:, blk * G : (blk + 1) * G],
                        start=(blk == 0),
                        stop=(blk == 3),
                    )
                d["qcol"] = qcol

            def st11(t):  # qcol PSUM -> SBUF on ACT
                d = tctx[t]
                d["qcol_sb"] = qpool.tile([128, G], f16, name="qcol_sb")
                nc.scalar.copy(out=d["qcol_sb"], in_=d["qcol"])

            def st12(t):  # qT = qcol.T on PE
                if t >= N_FULL - 6:
                    return  # drain tiles use the Pool-only out-add
                d = tctx[t]
                qT = ps_qt.tile([8, 128], f16)
                nc.tensor.transpose(out=qT, in_=d["qcol_sb"], identity=ident_sb)
                d["qT"] = qT

            def st13(t):  # qT PSUM -> SBUF on ACT
                if t >= N_FULL - 6:
                    return
                d = tctx[t]
                d["qT_sb"] = qpool.tile([8, 128], f16, name="qT_sb")
                nc.scalar.copy(out=d["qT_sb"], in_=d["qT"])

            def st14(t):  # groups 0..4: scores into PSUM + q broadcast on PE
                if t >= N_FULL - 6:
                    return
                d = tctx[t]
                out_ps = ps_o.tile([128, PEG * 80], f32)
                scoresf = d["scores_t"].rearrange("p g d -> p (g d)")
                nc.tensor.matmul(
                    out=out_ps,
                    lhsT=ident_sb,
                    rhs=scoresf[:, 0 : PEG * 80],
                    start=True,
                    stop=False,
                )
                nc.tensor.matmul(
                    out=out_ps,
                    lhsT=d["qT_sb"],
                    rhs=mask_sb[:, 0 : PEG * 80],
                    start=False,
                    stop=True,
                )
                d["out_ps"] = out_ps

            def st15(t):  # out_sb: ACT copies groups 0..4, Pool adds 5..7
                d = tctx[t]
                out_sb = opool.tile([128, G, 80], f16)
                if t >= N_FULL - 6:
                    # drain tiles: DVE is idle past the last max8
                    nc.vector.tensor_tensor(
                        out=out_sb,
                        in0=d["scores_t"],
                        in1=d["qcol_sb"][:].unsqueeze(2).broadcast_to(
                            [128, G, 80]
                        ),
                        op=mybir.AluOpType.add,
                    )
                    d["out_sb"] = out_sb
                    return
                nc.scalar.copy(
                    out=out_sb.rearrange("p g d -> p (g d)")[:, 0 : PEG * 80],
                    in_=d["out_ps"],
                )
                nc.gpsimd.tensor_tensor(
                    out=out_sb[:, PEG:G, :],
                    in0=d["scores_t"][:, PEG:G, :],
                    in1=d["qcol_sb"][:, PEG:G].unsqueeze(2).broadcast_to(
                        [128, G - PEG, 80]
                    ),
                    op=mybir.AluOpType.add,
                )
                d["out_sb"] = out_sb

            def st16(t):  # store on SP
                d = tctx.pop(t)
                r0 = t * ROWS_PER_TILE
                nc.sync.dma_start(
                    out=out[r0 : r0 + ROWS_PER_TILE, :].rearrange(
                        "(p g) d -> p g d", p=128
                    ),
                    in_=d["out_sb"],
                )

            # ---- 64-row tail tile: 64 partitions x 1 row-group, stages
            # interleaved into the main pipeline ticks ----
            tl = {}

            def tl0():
                r0 = N_FULL * ROWS_PER_TILE
                tl["pred_tl"] = pin.tile([64, C * NB], f16, name="pred_tl")
                nc.sync.dma_start(out=tl["pred_tl"], in_=pred[r0 : r0 + TAIL, :])
                tl["scores_tl"] = sin.tile([64, 80], f16, name="scores_tl")
                nc.sync.dma_start(
                    out=tl["scores_tl"], in_=scores[r0 : r0 + TAIL, :]
                )

            def tl1():
                tl["e_tl"] = epool.tile([64, C * NB], f16, name="e_tl")
                nc.scalar.activation(
                    out=tl["e_tl"],
                    in_=tl["pred_tl"],
                    func=mybir.ActivationFunctionType.Exp,
                )

            def tl2():
                e_tl = tl["e_tl"]
                t8_tl = tpool.tile([64, C, 8], f16)
                t8_tlf = t8_tl.rearrange("p c k -> p (c k)")
                for i in range(C):
                    nc.vector.max(
                        out=t8_tlf[:, i * 8 : i * 8 + 8],
                        in_=e_tl[:, i * NB : (i + 1) * NB],
                    )
                s4 = small.tile([64, C], f32)
                nc.vector.tensor_reduce(
                    out=s4,
                    in_=e_tl.rearrange("p (c b) -> p c b", b=NB),
                    axis=mybir.AxisListType.X,
                    op=mybir.AluOpType.add,
                )
                tl["t8_tl"] = t8_tl
                tl["s4"] = s4

            def tl3():
                r4 = small.tile([64, C], f16, name="r4")
                with nc.allow_low_precision(reason="fp16 stat tolerates 5e-4"):
                    nc.vector.reciprocal(out=r4, in_=tl["s4"])
                stat_tl = statp.tile([64, C, K], f16)
                nc.vector.tensor_tensor(
                    out=stat_tl,
                    in0=tl["t8_tl"][:, :, 0:K],
                    in1=r4[:].unsqueeze(2).broadcast_to([64, C, K]),
                    op=mybir.AluOpType.mult,
                )
                statT = ps_t.tile([64, 256], f16)
                nc.tensor.transpose(
                    out=statT[0:16, 0:64],
                    in_=stat_tl.rearrange("p c k -> p (c k)"),
                    identity=ident_sb[0:64, 0:64],
                )
                tl["statT"] = statT

            def tl4():
                tl["statT_tl_sb"] = stp.tile([16, 64], f16, name="statT_tl_sb")
                nc.scalar.copy(
                    out=tl["statT_tl_sb"], in_=tl["statT"][0:16, 0:64]
                )
                # w1pair rows 0..15, cols 0..63 are exactly W1eff (subgroup 0)
                hT = ps_h.tile([128, 4 * 128], f32)
                nc.tensor.matmul(
                    out=hT[0:64, 0:64],
                    lhsT=w1_sb[0:16, 0:64],
                    rhs=tl["statT_tl_sb"],
                    start=True,
                    stop=True,
                )
                tl["hT"] = hT

            def tl5():
                tl["hT_tl_sb"] = hpool.tile([64, 64], f16, name="hT_tl_sb")
                nc.scalar.activation(
                    out=tl["hT_tl_sb"],
                    in_=tl["hT"][0:64, 0:64],
                    func=mybir.ActivationFunctionType.Relu,
                    bias=b1_sb[0:64, :],
                    scale=1.0,
                )
                # q as a column [64, 1]: contraction over the 64 tail hiddens
                qcol = ps_q.tile([128, G], f32)
                nc.tensor.matmul(
                    out=qcol[0:64, 0:1],
                    lhsT=tl["hT_tl_sb"],
                    rhs=w2_sb[0:64, 0:1],
                    start=True,
                    stop=True,
                )
                tl["qcol"] = qcol

            def tl6():
                tl["qc2_sb"] = qpool.tile([64, 1], f16, name="qc2_sb")
                nc.scalar.copy(out=tl["qc2_sb"], in_=tl["qcol"][0:64, 0:1])

            def tl7():
                out_sb_tl = opool.tile([64, 80], f16)
                nc.gpsimd.tensor_tensor(
                    out=out_sb_tl,
                    in0=tl["scores_tl"],
                    in1=tl["qc2_sb"][:].broadcast_to([64, 80]),
                    op=mybir.AluOpType.add,
                )
                r0 = N_FULL * ROWS_PER_TILE
                nc.sync.dma_start(out=out[r0 : r0 + TAIL, :], in_=out_sb_tl)

            tail_stages = [tl0, tl1, tl2, tl3, tl4, tl5, tl6, tl7]
            stages = [st0, st1, st2, st3, st4, st5, st6, st7, st8, st9,
                      st10, st11, st12, st13, st14, st15, st16]
            TAIL_BASE = 20

            for tick in range(N_FULL + len(stages) - 1):
                for si in range(len(stages) - 1, -1, -1):
                    t = tick - si
                    if 0 <= t < N_FULL:
                        stages[si](t)
                if (
                    TAIL_BASE <= tick < TAIL_BASE + 3 * len(tail_stages)
                    and (tick - TAIL_BASE) % 3 == 0
                ):
                    tail_stages[(tick - TAIL_BASE) // 3]()
    nc.compile()
    return nc


def _get_nc(rows):
    if rows not in _CACHE:
        _CACHE[rows] = _build(rows)
    return _CACHE[rows]


def _prep_host(scores, pred_corners, W1, b1, W2, b2):
    B, L, c, nb = pred_corners.shape
    BL = B * L
    scores_f = (
        scores.reshape(BL, scores.shape[-1]).astype(np.float32) + np.float32(b2[0])
    ).astype(np.float16)
    pred_f = np.ascontiguousarray(pred_corners.reshape(BL, c * nb), dtype=np.float16)
    idx = [ci * (K + 1) + k for ci in range(C) for k in range(K)]
    W1eff = (W1[idx] + 0.25 * np.repeat(W1[K :: K + 1], K, axis=0)).astype(np.float32)
    # w1pair[:, pair*128:(pair+1)*128]: 128 cols = [hidden of group 2*pair |
    # hidden of group 2*pair+1]; row j contributes iff j//16 == that group's
    # subgroup index.
    w1pair = np.zeros((64, 256), np.float32)
    for pairidx in range(2):
        for tb in range(2):
            gp = pairidx * 2 + tb
            w1pair[
                gp * 16 : (gp + 1) * 16,
                pairidx * 128 + tb * 64 : pairidx * 128 + (tb + 1) * 64,
            ] = W1eff
    w2q = np.zeros((128, 32), np.float32)
    for blk in range(4):
        for hh in range(128):
            g = (blk // 2) * 4 + (blk % 2) * 2 + hh // 64
            w2q[hh, blk * 8 + g] = W2[hh % 64, 0]
    b1r = np.tile(b1.astype(np.float32).reshape(HID, 1), (2, 1))
    ident = np.eye(128, dtype=np.float16)
    mask8 = np.zeros((8, 640), np.float32)
    for g in range(8):
        mask8[g, g * 80 : (g + 1) * 80] = 1.0
    return (
        scores_f,
        pred_f,
        w1pair.astype(np.float16),
        w2q.astype(np.float16),
        b1r,
        ident,
        mask8.astype(np.float16),
    )


def _run(scores, pred_corners, W1, b1, W2, b2, trace=False):
    from concourse.bass_utils import run_bass_kernel_spmd

    B, L, _, _ = pred_corners.shape
    scores_f, pred_f, w1pair, w2q, b1r, ident, mask8 = _prep_host(
        scores, pred_corners, W1, b1, W2, b2
    )
    nc = _get_nc(ROWS_PER_CORE)
    in_maps = []
    for i in range(N_CORES):
        lo = i * ROWS_PER_CORE
        hi = lo + ROWS_PER_CORE
        in_maps.append(
            {
                "pred": pred_f[lo:hi],
                "scores": scores_f[lo:hi],
                "w1pair": w1pair,
                "w2q": w2q,
                "b1r": b1r,
                "ident": ident,
                "mask8": mask8,
            }
        )
    kwargs = {}
    if trace:
        kwargs = dict(trace=True, trace_cores=list(range(N_CORES)))
    res = run_bass_kernel_spmd(nc, in_maps, core_ids=list(range(N_CORES)), **kwargs)
    parts = [res.results[i]["out"] for i in range(N_CORES)]
    full = np.concatenate(parts, axis=0).astype(np.float32).reshape(B, L, 80)
    return full, res


def kernel(scores, pred_corners, W1, b1, W2, b2):
    full, _ = _run(
        np.asarray(scores),
        np.asarray(pred_corners),
        np.asarray(W1),
        np.asarray(b1),
        np.asarray(W2),
        np.asarray(b2),
    )
    return full


# revision 44
# speedup vs baseline: 1.0296x; 1.0296x over previous
"""Trainium2 Bass kernel for nn_LQE (topk_masking).

out = scores + MLP(topk_softmax_stats(pred_corners))

Math notes:
- top_k(softmax(x)) == softmax values of top_k(x) (softmax is monotone), and
  exp(x) is safe unnormalized here (|x| <~ 6 for randn inputs), so
  p_k = exp(x)_k / sum(exp(x)).  DVE `max` returns top-8 sorted descending in
  one instruction -> top-4 directly.
- The mean-of-top4 stat feature folds into W1 on the host:
  W1eff[c*4+k] = W1[c*5+k] + 0.25*W1[c*5+4]; b2 folds into scores.

Engine budget per 1024-row supertile (steady state ~3.2us):
- DVE: the 32 max8 instructions (~3.0us, the bottleneck) + reciprocal of the
  denominators + the top4 * (1/s) scale.
- Pool (GPSIMD): the denominator pairwise add-tree (TensorTensor add is the
  only legal elementwise reduce on Pool) + 3 groups of the final
  scores+quality broadcast add.
- ACT: exp / relu / PSUM->SBUF copies.
- PE: stat transposes, both MLP layers, and scores+quality for 5 groups via
  PSUM accumulation (identity matmul + one-hot broadcast matmul).
All device I/O is fp16 to halve DMA_ENGINES occupancy; the fp16 rounding is
~5e-4 relative, far inside the 2e-2 gate.

The 39 supertiles are emitted as a 17-stage software pipeline with exactly
one cross-engine hop per stage boundary: at steady state every instruction's
producers completed >=1 tick earlier, so the in-order sequencers never
head-block (DVE streams max8 back-to-back).  A 64-row tail tile handles
40000 = 39*1024 + 64 exactly, its stages interleaved into the main ticks.

Sharding: pure data-parallel over B*L = 320000 rows, 40000 rows/core on 8
cores (row = 8*p + g within a supertile).
"""

import numpy as np

K = 4
C = 4
NB = 33
HID = 64
G = 8  # row-groups per partition per supertile
ROWS_PER_TILE = 128 * G  # 1024
N_CORES = 8
ROWS_PER_CORE = 40000
N_FULL = 39  # 39 * 1024 = 39936
TAIL = 64  # + 64 = 40000

_CACHE = {}


def _build(rows):
    import concourse.bacc as bacc
    import concourse.mybir as mybir
    from concourse.tile import TileContext

    f32 = mybir.dt.float32
    f16 = mybir.dt.float16
    assert rows == N_FULL * ROWS_PER_TILE + TAIL

    nc = bacc.Bacc("TRN2")
    pred = nc.dram_tensor("pred", [rows, C * NB], f16, kind="ExternalInput")
    scores = nc.dram_tensor("scores", [rows, 80], f16, kind="ExternalInput")
    # w1pair[:, pair*128 + col]: col<64 -> hidden col of even group of pair,
    # col>=64 -> hidden of odd group; rows j in [0,64): j//16 = subgroup,
    # j%16 = stat feature (c*4+k).  Zero rows select the right subgroup.
    w1pair = nc.dram_tensor("w1pair", [64, 256], f16, kind="ExternalInput")
    # w2q[hh, blk*8+j] = W2[hh%64] iff j == (blk//2)*4 + (blk%2)*2 + hh//64:
    # one-hot columns so 4 accumulating matmuls produce qcol[p, g] from the
    # 2-group-stacked hT.  Column 0 of block 0 (rows 0..63) is plain W2,
    # reused by the tail tile.
    w2q = nc.dram_tensor("w2q", [128, 32], f16, kind="ExternalInput")
    b1r = nc.dram_tensor("b1r", [128, 1], f32, kind="ExternalInput")
    ident = nc.dram_tensor("ident", [128, 128], f16, kind="ExternalInput")
    # mask8[g, j] = (j // 80 == g), for the one-hot broadcast matmul.
    mask8 = nc.dram_tensor("mask8", [8, 640], f16, kind="ExternalInput")
    out = nc.dram_tensor("out", [rows, 80], f16, kind="ExternalOutput")

    NG = G * C  # 32 softmax groups per partition
    PEG = 5  # groups 0..4 take the PE out-add path; 5..7 go to Pool

    with TileContext(nc) as tc:
        with (
            tc.tile_pool(name="singles", bufs=1) as singles,
            tc.tile_pool(name="pin", bufs=6) as pin,
            tc.tile_pool(name="sin", bufs=19) as sin,
            tc.tile_pool(name="epool", bufs=6) as epool,
            tc.tile_pool(name="tpool", bufs=6) as tpool,
            tc.tile_pool(name="small", bufs=5) as small,
            tc.tile_pool(name="statp", bufs=6) as statp,
            tc.tile_pool(name="stp", bufs=4) as stp,
            tc.tile_pool(name="hpool", bufs=4) as hpool,
            tc.tile_pool(name="qpool", bufs=6) as qpool,
            tc.tile_pool(name="opool", bufs=4) as opool,
            tc.tile_pool(name="ps_t", bufs=2, space="PSUM") as ps_t,
            tc.tile_pool(name="ps_h", bufs=2, space="PSUM") as ps_h,
            tc.tile_pool(name="ps_q", bufs=1, space="PSUM") as ps_q,
            tc.tile_pool(name="ps_qt", bufs=1, space="PSUM") as ps_qt,
            tc.tile_pool(name="ps_o", bufs=2, space="PSUM") as ps_o,
        ):
            w1_sb = singles.tile([64, 256], f16)
            nc.sync.dma_start(out=w1_sb, in_=w1pair[:, :])
            w2_sb = singles.tile([128, 32], f16)
            nc.sync.dma_start(out=w2_sb, in_=w2q[:, :])
            b1_sb = singles.tile([128, 1], f32)
            nc.sync.dma_start(out=b1_sb, in_=b1r[:, :])
            ident_sb = singles.tile([128, 128], f16)
            nc.sync.dma_start(out=ident_sb, in_=ident[:, :])
            mask_sb = singles.tile([8, 640], f16)
            nc.sync.dma_start(out=mask_sb, in_=mask8[:, :])

            tctx = {}

            def st0(t):  # loads (row = G*p + g) on SP
                r0 = t * ROWS_PER_TILE
                d = {}
                hp = tc.high_priority()
                hp.__enter__()
                d["pred_t"] = pin.tile([128, G, C * NB], f16, name="pred_t")
                nc.sync.dma_start(
                    out=d["pred_t"],
                    in_=pred[r0 : r0 + ROWS_PER_TILE, :].rearrange(
                        "(p g) d -> p g d", p=128
                    ),
                )
                d["scores_t"] = sin.tile([128, G, 80], f16, name="scores_t")
                nc.sync.dma_start(
                    out=d["scores_t"],
                    in_=scores[r0 : r0 + ROWS_PER_TILE, :].rearrange(
                        "(p g) d -> p g d", p=128
                    ),
                )
                hp.__exit__(None, None, None)
                tctx[t] = d

            def st1(t):  # e = exp(pred) on ACT (fp16 out)
                d = tctx[t]
                d["e"] = epool.tile([128, G, C * NB], f16, name="e")
                with tc.high_priority():
                    nc.scalar.activation(
                        out=d["e"],
                        in_=d["pred_t"],
                        func=mybir.ActivationFunctionType.Exp,
                    )

            def st2(t):  # top-8 per group on DVE; first denominator add
                # level (c16 = e[0:16] + e[16:32]) on Pool
                d = tctx[t]
                e = d["e"]
                ef = e.rearrange("p g d -> p (g d)")
                t8 = tpool.tile([128, NG, 8], f16)
                t8f = t8.rearrange("p i k -> p (i k)")
                for i in range(NG):
                    nc.vector.max(
                        out=t8f[:, i * 8 : i * 8 + 8],
                        in_=ef[:, i * NB : (i + 1) * NB],
                    )
                e3 = e.rearrange("p g (c b) -> p (g c) b", b=NB)
                c16 = small.tile([128, NG, 16], f16, name="c16")
                nc.gpsimd.tensor_tensor(
                    out=c16,
                    in0=e3[:, :, 0:16],
                    in1=e3[:, :, 16:32],
                    op=mybir.AluOpType.add,
                )
                d["t8"] = t8
                d["c16"] = c16
                d["e3"] = e3

            def st3(t):  # rest of the denominator add-tree on Pool
                d = tctx[t]
                c16 = d["c16"]
                c8 = small.tile([128, NG, 8], f16, name="c8")
                nc.gpsimd.tensor_tensor(
                    out=c8,
                    in0=c16[:, :, 0:8],
                    in1=c16[:, :, 8:16],
                    op=mybir.AluOpType.add,
                )
                c4 = small.tile([128, NG, 4], f16, name="c4")
                nc.gpsimd.tensor_tensor(
                    out=c4,
                    in0=c8[:, :, 0:4],
                    in1=c8[:, :, 4:8],
                    op=mybir.AluOpType.add,
                )
                c2 = small.tile([128, NG, 2], f16, name="c2")
                nc.gpsimd.tensor_tensor(
                    out=c2,
                    in0=c4[:, :, 0:2],
                    in1=c4[:, :, 2:4],
                    op=mybir.AluOpType.add,
                )
                s1 = small.tile([128, NG], f16, name="s1")
                nc.gpsimd.tensor_tensor(
                    out=s1,
                    in0=c2[:, :, 0],
                    in1=c2[:, :, 1],
                    op=mybir.AluOpType.add,
                )
                s32 = small.tile([128, NG], f16, name="s32")
                nc.gpsimd.tensor_tensor(
                    out=s32,
                    in0=s1,
                    in1=d["e3"][:, :, 32],
                    op=mybir.AluOpType.add,
                )
                d["s32"] = s32

            def st4(t):  # r32 = 1/s on DVE
                d = tctx[t]
                d["r32"] = small.tile([128, NG], f16, name="r32")
                with nc.allow_low_precision(reason="fp16 stat tolerates 5e-4"):
                    nc.vector.reciprocal(out=d["r32"], in_=d["s32"])

            def st5(t):  # stat = top4 * (1/s) on DVE (fp16 2x)
                d = tctx[t]
                stat = statp.tile([128, NG, K], f16)
                nc.vector.tensor_tensor(
                    out=stat,
                    in0=d["t8"][:, :, 0:K],
                    in1=d["r32"][:].unsqueeze(2).broadcast_to([128, NG, K]),
                    op=mybir.AluOpType.mult,
                )
                d["stat"] = stat

            def st6(t):  # statT transposes on PE (two 64-col halves)
                d = tctx[t]
                statf = d["stat"].rearrange("p i k -> p (i k)")
                statT = ps_t.tile([64, 256], f16)
                nc.tensor.transpose(
                    out=statT[:, 0:128], in_=statf[:, 0:64], identity=ident_sb
                )
                nc.tensor.transpose(
                    out=statT[:, 128:256], in_=statf[:, 64:128], identity=ident_sb
                )
                d["statT"] = statT

            def st7(t):  # statT PSUM -> SBUF on ACT
                d = tctx[t]
                d["statT_sb"] = stp.tile([64, 256], f16, name="statT_sb")
                nc.scalar.copy(out=d["statT_sb"], in_=d["statT"])

            def st8(t):  # MLP layer 1 on PE, two groups stacked per matmul
                # hT[hh, blk*128+p]: group = (blk//2)*4 + (blk%2)*2 + hh//64.
                d = tctx[t]
                hT = ps_h.tile([128, 4 * 128], f32)
                for half in range(2):
                    for pairidx in range(2):
                        blk = half * 2 + pairidx
                        nc.tensor.matmul(
                            out=hT[:, blk * 128 : (blk + 1) * 128],
                            lhsT=w1_sb[:, pairidx * 128 : (pairidx + 1) * 128],
                            rhs=d["statT_sb"][:, half * 128 : (half + 1) * 128],
                            start=True,
                            stop=True,
                        )
                d["hT"] = hT

            def st9(t):  # relu + b1 on ACT
                d = tctx[t]
                d["hT_sb"] = hpool.tile([128, 4 * 128], f16, name="hT_sb")
                nc.scalar.activation(
                    out=d["hT_sb"],
                    in_=d["hT"],
                    func=mybir.ActivationFunctionType.Relu,
                    bias=b1_sb,
                    scale=1.0,
                )

            def st10(t):  # MLP layer 2 on PE: qcol[p, g] via one-hot accum
                d = tctx[t]
                qcol = ps_q.tile([128, G], f32)
                for blk in range(4):
                    nc.tensor.matmul(
                        out=qcol,
                        lhsT=d["hT_sb"][:, blk * 128 : (blk + 1) * 128],
                        rhs=w2_sb[:, blk * G : (blk + 1) * G],
                        start=(blk == 0),
                        stop=(blk == 3),
                    )
                d["qcol"] = qcol

            def st11(t):  # qcol PSUM -> SBUF on ACT
                d = tctx[t]
                d["qcol_sb"] = qpool.tile([128, G], f16, name="qcol_sb")
                nc.scalar.copy(out=d["qcol_sb"], in_=d["qcol"])

            def st12(t):  # qT = qcol.T on PE
                if t >= N_FULL - 6:
                    return  # drain tiles use the Pool-only out-add
                d = tctx[t]
                qT = ps_qt.tile([8, 128], f16)
                nc.tensor.transpose(out=qT, in_=d["qcol_sb"], identity=ident_sb)
                d["qT"] = qT

            def st13(t):  # qT PSUM -> SBUF on ACT
                if t >= N_FULL - 6:
                    return
                d = tctx[t]
                d["qT_sb"] = qpool.tile([8, 128], f16, name="qT_sb")
                nc.scalar.copy(out=d["qT_sb"], in_=d["qT"])

            def st14(t):  # groups 0..4: scores into PSUM + q broadcast on PE
                if t >= N_FULL - 6:
                    return
                d = tctx[t]
                out_ps = ps_o.tile([128, PEG * 80], f32)
                scoresf = d["scores_t"].rearrange("p g d -> p (g d)")
                nc.tensor.matmul(
                    out=out_ps,
                    lhsT=ident_sb,
                    rhs=scoresf[:, 0 : PEG * 80],
                    start=True,
                    stop=False,
                )
                nc.tensor.matmul(
                    out=out_ps,
                    lhsT=d["qT_sb"],
                    rhs=mask_sb[:, 0 : PEG * 80],
                    start=False,
                    stop=True,
                )
                d["out_ps"] = out_ps

            def st15(t):  # out_sb: ACT copies groups 0..4, Pool adds 5..7
                d = tctx[t]
                out_sb = opool.tile([128, G, 80], f16)
                if t >= N_FULL - 6:
                    # drain tiles: DVE is idle past the last max8
                    nc.vector.tensor_tensor(
                        out=out_sb,
                        in0=d["scores_t"],
                        in1=d["qcol_sb"][:].unsqueeze(2).broadcast_to(
                            [128, G, 80]
                        ),
                        op=mybir.AluOpType.add,
                    )
                    d["out_sb"] = out_sb
                    return
                nc.scalar.copy(
                    out=out_sb.rearrange("p g d -> p (g d)")[:, 0 : PEG * 80],
                    in_=d["out_ps"],
                )
                nc.gpsimd.tensor_tensor(
                    out=out_sb[:, PEG:G, :],
                    in0=d["scores_t"][:, PEG:G, :],
                    in1=d["qcol_sb"][:, PEG:G].unsqueeze(2).broadcast_to(
                        [128, G - PEG, 80]
                    ),
                    op=mybir.AluOpType.add,
                )
                d["out_sb"] = out_sb

            def st16(t):  # store on SP
                d = tctx.pop(t)
                r0 = t * ROWS_PER_TILE
                nc.sync.dma_start(
                    out=out[r0 : r0 + ROWS_PER_TILE, :].rearrange(
                        "(p g) d -> p g d", p=128
                    ),
                    in_=d["out_sb"],
                )

            # ---- 64-row tail tile: 64 partitions x 1 row-group, stages
            # interleaved into the main pipeline ticks ----
            tl = {}

            def tl0():
                r0 = N_FULL * ROWS_PER_TILE
                tl["pred_tl"] = pin.tile([64, C * NB], f16, name="pred_tl")
                nc.sync.dma_start(out=tl["pred_tl"], in_=pred[r0 : r0 + TAIL, :])
                tl["scores_tl"] = sin.tile([64, 80], f16, name="scores_tl")
                nc.sync.dma_start(
                    out=tl["scores_tl"], in_=scores[r0 : r0 + TAIL, :]
                )

            def tl1():
                tl["e_tl"] = epool.tile([64, C * NB], f16, name="e_tl")
                nc.scalar.activation(
                    out=tl["e_tl"],
                    in_=tl["pred_tl"],
                    func=mybir.ActivationFunctionType.Exp,
                )

            def tl2():
                e_tl = tl["e_tl"]
                t8_tl = tpool.tile([64, C, 8], f16)
                t8_tlf = t8_tl.rearrange("p c k -> p (c k)")
                for i in range(C):
                    nc.vector.max(
                        out=t8_tlf[:, i * 8 : i * 8 + 8],
                        in_=e_tl[:, i * NB : (i + 1) * NB],
                    )
                s4 = small.tile([64, C], f32)
                nc.vector.tensor_reduce(
                    out=s4,
                    in_=e_tl.rearrange("p (c b) -> p c b", b=NB),
                    axis=mybir.AxisListType.X,
                    op=mybir.AluOpType.add,
                )
                tl["t8_tl"] = t8_tl
                tl["s4"] = s4

            def tl3():
                r4 = small.tile([64, C], f16, name="r4")
                with nc.allow_low_precision(reason="fp16 stat tolerates 5e-4"):
                    nc.vector.reciprocal(out=r4, in_=tl["s4"])
                stat_tl = statp.tile([64, C, K], f16)
                nc.vector.tensor_tensor(
                    out=stat_tl,
                    in0=tl["t8_tl"][:, :, 0:K],
                    in1=r4[:].unsqueeze(2).broadcast_to([64, C, K]),
                    op=mybir.AluOpType.mult,
                )
                statT = ps_t.tile([64, 256], f16)
                nc.tensor.transpose(
                    out=statT[0:16, 0:64],
                    in_=stat_tl.rearrange("p c k -> p (c k)"),
                    identity=ident_sb[0:64, 0:64],
                )
                tl["statT"] = statT

            def tl4():
                tl["statT_tl_sb"] = stp.tile([16, 64], f16, name="statT_tl_sb")
                nc.scalar.copy(
                    out=tl["statT_tl_sb"], in_=tl["statT"][0:16, 0:64]
                )
                # w1pair rows 0..15, cols 0..63 are exactly W1eff (subgroup 0)
                hT = ps_h.tile([128, 4 * 128], f32)
                nc.tensor.matmul(
                    out=hT[0:64, 0:64],
                    lhsT=w1_sb[0:16, 0:64],
                    rhs=tl["statT_tl_sb"],
                    start=True,
                    stop=True,
                )
                tl["hT"] = hT

            def tl5():
                tl["hT_tl_sb"] = hpool.tile([64, 64], f16, name="hT_tl_sb")
                nc.scalar.activation(
                    out=tl["hT_tl_sb"],
                    in_=tl["hT"][0:64, 0:64],
                    func=mybir.ActivationFunctionType.Relu,
                    bias=b1_sb[0:64, :],
                    scale=1.0,
                )
                # q as a column [64, 1]: contraction over the 64 tail hiddens
                qcol = ps_q.tile([128, G], f32)
                nc.tensor.matmul(
                    out=qcol[0:64, 0:1],
                    lhsT=tl["hT_tl_sb"],
                    rhs=w2_sb[0:64, 0:1],
                    start=True,
                    stop=True,
                )
                tl["qcol"] = qcol

            def tl6():
                tl["qc2_sb"] = qpool.tile([64, 1], f16, name="qc2_sb")
                nc.scalar.copy(out=tl["qc2_sb"], in_=tl["qcol"][0:64, 0:1])

            def tl7():
                out_sb_tl = opool.tile([64, 80], f16)
                nc.gpsimd.tensor_tensor(
                    out=out_sb_tl,
                    in0=tl["scores_tl"],
                    in1=tl["qc2_sb"][:].broadcast_to([64, 80]),
                    op=mybir.AluOpType.add,
                )
                r0 = N_FULL * ROWS_PER_TILE
                nc.sync.dma_start(out=out[r0 : r0 + TAIL, :], in_=out_sb_tl)

            tail_stages = [tl0, tl1, tl2, tl3, tl4, tl5, tl6, tl7]
            stages = [st0, st1, st2, st3, st4, st5, st6, st7, st8, st9,
                      st10, st11, st12, st13, st14, st15, st16]
            TAIL_BASE = 20

            for tick in range(N_FULL + len(stages) - 1):
                for si in range(len(stages) - 1, -1, -1):
                    t = tick - si
                    if 0 <= t < N_FULL:
                        stages[si](t)
                if (
                    TAIL_BASE <= tick < TAIL_BASE + 3 * len(tail_stages)
                    and (tick - TAIL_BASE) % 3 == 0
                ):
                    tail_stages[(tick - TAIL_BASE) // 3]()
    nc.compile()
    return nc


def _get_nc(rows):
    if rows not in _CACHE:
        _CACHE[rows] = _build(rows)
    return _CACHE[rows]


def _prep_host(scores, pred_corners, W1, b1, W2, b2):
    B, L, c, nb = pred_corners.shape
    BL = B * L
    scores_f = (
        scores.reshape(BL, scores.shape[-1]).astype(np.float32) + np.float32(b2[0])
    ).astype(np.float16)
    pred_f = np.ascontiguousarray(pred_corners.reshape(BL, c * nb), dtype=np.float16)
    idx = [ci * (K + 1) + k for ci in range(C) for k in range(K)]
    W1eff = (W1[idx] + 0.25 * np.repeat(W1[K :: K + 1], K, axis=0)).astype(np.float32)
    # w1pair[:, pair*128:(pair+1)*128]: 128 cols = [hidden of group 2*pair |
    # hidden of group 2*pair+1]; row j contributes iff j//16 == that group's
    # subgroup index.
    w1pair = np.zeros((64, 256), np.float32)
    for pairidx in range(2):
        for tb in range(2):
            gp = pairidx * 2 + tb
            w1pair[
                gp * 16 : (gp + 1) * 16,
                pairidx * 128 + tb * 64 : pairidx * 128 + (tb + 1) * 64,
            ] = W1eff
    w2q = np.zeros((128, 32), np.float32)
    for blk in range(4):
        for hh in range(128):
            g = (blk // 2) * 4 + (blk % 2) * 2 + hh // 64
            w2q[hh, blk * 8 + g] = W2[hh % 64, 0]
    b1r = np.tile(b1.astype(np.float32).reshape(HID, 1), (2, 1))
    ident = np.eye(128, dtype=np.float16)
    mask8 = np.zeros((8, 640), np.float32)
    for g in range(8):
        mask8[g, g * 80 : (g + 1) * 80] = 1.0
    return (
        scores_f,
        pred_f,
        w1pair.astype(np.float16),
        w2q.astype(np.float16),
        b1r,
        ident,
        mask8.astype(np.float16),
    )


def _run(scores, pred_corners, W1, b1, W2, b2, trace=False):
    from concourse.bass_utils import run_bass_kernel_spmd

    B, L, _, _ = pred_corners.shape
    scores_f, pred_f, w1pair, w2q, b1r, ident, mask8 = _prep_host(
        scores, pred_corners, W1, b1, W2, b2
    )
    nc = _get_nc(ROWS_PER_CORE)
    in_maps = []
    for i in range(N_CORES):
        lo = i * ROWS_PER_CORE
        hi = lo + ROWS_PER_CORE
        in_maps.append(
            {
                "pred": pred_f[lo:hi],
                "scores": scores_f[lo:hi],
                "w1pair": w1pair,
                "w2q": w2q,
                "b1r": b1r,
                "ident": ident,
                "mask8": mask8,
            }
        )
    kwargs = {}
    if trace:
        kwargs = dict(trace=True, trace_cores=list(range(N_CORES)))
    res = run_bass_kernel_spmd(nc, in_maps, core_ids=list(range(N_CORES)), **kwargs)
    parts = [res.results[i]["out"] for i in range(N_CORES)]
    full = np.concatenate(parts, axis=0).astype(np.float32).reshape(B, L, 80)
    return full, res


def kernel(scores, pred_corners, W1, b1, W2, b2):
    full, _ = _run(
        np.asarray(scores),
        np.asarray(pred_corners),
        np.asarray(W1),
        np.asarray(b1),
        np.asarray(W2),
        np.asarray(b2),
    )
    return full
